# revision 1
# baseline (speedup 1.0000x reference)
"""Trainium2 Bass kernel for nn_Encoder_66872640799015 (segment_reduce).

Recurrent conv encoder over 32768 pedestrians (4096 scenes x 8), 12 steps.
Sharding: data-parallel over scenes — 8 cores x 4096 pedestrians (512 whole
scenes per core), weights replicated.

Key algorithmic idea: each scan step shifts the conv input window by one
column, so all conv outputs except the newest position roll over from the
previous step.  Per step only ONE new conv position per layer is computed
(~4.8x FLOP reduction).  Rolling ring buffers live in SBUF for the whole
kernel; weight matrices are pre-permuted on the host for each of the 3 ring
rotations so no data movement is needed for the rolling.

Layout: channel-major (channels on partitions, pedestrians on the free dim).
Matmuls run as float32r (full-rate fp32 storage) for the conv stack and
bfloat16 for the small dec / rel heads.  The per-scene segment max is a
strided VectorE reduce over groups of 8 along the free dim.  The final
rels output for all 12 steps is computed at the end as a single M=24
block-matmul over the stored conv3 features.
"""

import sys

sys.path.insert(0, "/opt/trn_rl_repo")

import numpy as np
import ml_dtypes

import concourse.bass as bass
import concourse.bacc as bacc
import concourse.tile as tile
from concourse import mybir
from concourse.bass_utils import run_bass_kernel_spmd

NCORES = 8
BATCH = 32768
B = BATCH // NCORES        # pedestrians per core
T = 8                      # obs_len
SEQ = 12                   # seq_len
SCENE = 8                  # pedestrians per scene
NS = B // SCENE            # scenes per core
CH = 512                   # free-dim chunk (one PSUM bank of fp32)
NCHUNK = B // CH
NSLOT = SEQ // 2           # S_all free slots (2 steps per slot)

F32 = mybir.dt.float32
F32R = mybir.dt.float32r
BF16 = mybir.dt.bfloat16

_cache = {}

# engine-assignment knobs (chunks 0..NCHUNK-1): chunk < knob -> ScalarE(ACT),
# else VectorE(DVE)
R3_ACT = 8     # relu3-lo split
R2_ACT = 8     # relu2 split
R1_ACT = 8     # relu1 split
S1_ACT = 0     # stage-1 copy split
SEG_POOL = False   # segment-max on GpSimd instead of VectorE
HI_POOL = False    # conv3 hi-band dup copy on GpSimd
PSUM_BUFS = (3, 2, 1, 1)   # (pdec, pc1, pc2, pc3); wide bufs cost 2 banks
WIDE1 = False   # conv1 psum spans 2 banks -> one relu per chunk-pair
WIDE2 = True    # conv2 psum spans 2 banks -> one relu per chunk-pair


def _perm(r):
    """S-feature row (32*t + ch) -> reference feature index (2*ch + t)."""
    t, ch = r // 32, r % 32
    return 2 * ch + t


def _host_weights(W_se, b_se, v1, g1, b1, v2, g2, b2, v3, g3, b3, W_hp, b_hp):
    """Derive all device weight tensors (pre-permuted / rotation variants)."""
    f32 = np.float32

    def wn(v, g):
        n = np.sqrt((v * v).sum(axis=(1, 2)))
        return (v * (g / n)[:, None, None]).astype(f32)

    w1 = wn(v1, g1)   # (64, 64, 3)
    w2 = wn(v2, g2)   # (32, 64, 3)
    w3 = wn(v3, g3)   # (32, 32, 3)

    # conv lhsT rotation variants.  Ring slot j holds tap k = (j - r) mod 3
    # where r is the rotation (= conv position mod 3).
    def conv_variants(w, nin, nout, nslots):
        # returns (nslots*nin, 3, nout): [slot-block rows, rotation, out]
        out = np.zeros((nslots * nin, 3, nout), f32)
        for r in range(3):
            for j in range(nslots):
                k = (j - r) % 3
                # lhsT rows = input channels of slot j, cols = out channel
                out[j * nin:(j + 1) * nin, r, :] = w[:, :, k].T
        return out

    w1A = conv_variants(w1, 64, 64, 2)            # (128, 3, 64) slots 0,1
    w1C = conv_variants(w1, 64, 64, 3)[128:]      # (64, 3, 64)  slot 2
    w2A = conv_variants(w2, 64, 32, 2)            # (128, 3, 32)
    w2C = conv_variants(w2, 64, 32, 3)[128:]      # (64, 3, 32)
    w3A = conv_variants(w3, 32, 32, 3)            # (96, 3, 32)
    # bias rows: ring tiles carry a constant ones-row as an extra partition,
    # so the conv bias rides in the matmul (lhsT bottom row) and the relus
    # become bias-free single ops placeable on either engine.
    w1C = np.concatenate([w1C, np.tile(b1.reshape(1, 1, 64), (1, 3, 1))], 0)
    w2C = np.concatenate([w2C, np.tile(b2.reshape(1, 1, 32), (1, 3, 1))], 0)
    w3A = np.concatenate([w3A, np.tile(b3.reshape(1, 1, 32), (1, 3, 1))], 0)

    perm = np.array([_perm(r) for r in range(64)])

    # dec = A_mat @ s + Bm_mat @ mx[seg] + c_d   (feedback column, 64-dim)
    W_hpa, W_hpb = W_hp[:, :64], W_hp[:, 64:]
    A_mat = (W_se @ W_hpa).astype(f32)    # (64 emb, 64 feat)
    Bm_mat = (W_se @ W_hpb).astype(f32)
    c_d = (W_se @ b_hp + b_se).astype(f32)
    # doubled vertically so lhsT can be sliced at base partition 0 or 64
    # to match the S/MX band of even/odd steps (matmul requires equal
    # base_partition for lhsT and rhs)
    decA = np.vstack([A_mat[:, perm].T] * 2).copy()   # (128, 64)
    decB = np.vstack([Bm_mat[:, perm].T] * 2).copy()

    # rel endgame: out partition p = 2*k + c (k=step, c=coord).
    # lhsT per slot: (128 rows = [band0: step 2*slot, band1: step 2*slot+1]
    #                 feature rows, 24 cols)
    relA = np.zeros((128, NSLOT, 24), f32)
    relB = np.zeros((128, NSLOT, 24), f32)
    for slot in range(NSLOT):
        for band in range(2):
            k = 2 * slot + band
            rows = slice(band * 64, band * 64 + 64)
            for c in range(2):
                relA[rows, slot, 2 * k + c] = W_hpa[c, perm]
                relB[rows, slot, 2 * k + c] = W_hpb[c, perm]

    bf = ml_dtypes.bfloat16
    return {
        "wse_t": np.concatenate(
            [np.ascontiguousarray(W_se.T, f32), b_se.reshape(1, 64)], 0),
        "w1A": w1A.reshape(128, 3 * 64),
        "w1C": w1C.reshape(65, 3 * 64),
        "w2A": w2A.reshape(128, 3 * 32),
        "w2C": w2C.reshape(65, 3 * 32),
        "w3A": w3A.reshape(97, 3 * 32),
        "decA": decA.astype(bf),
        "decB": decB.astype(bf),
        "relA": relA.reshape(128, NSLOT * 24).astype(bf),
        "relB": relB.reshape(128, NSLOT * 24).astype(bf),
        "b_se": b_se.reshape(64, 1).astype(f32),
        "b_c1": b1.reshape(64, 1).astype(f32),
        "b_c2": b2.reshape(32, 1).astype(f32),
        "b_c3": b3.reshape(32, 1).astype(f32),
        "c_d": c_d.reshape(64, 1),
        "b_hp24": np.tile(b_hp.astype(f32), SEQ).reshape(24, 1),
        "ones": np.ones((1, B), f32),
    }


def _build_module():
    """Build the SPMD Bass module (input-independent, cached)."""
    nc = bacc.Bacc()

    obs_d = nc.dram_tensor("obs", [T, 3, B], F32R, kind="ExternalInput")
    wd = {}
    for name, p, f, dt in [
        ("wse_t", 3, 64, F32R), ("w1A", 128, 192, F32R), ("w1C", 65, 192, F32R),
        ("w2A", 128, 96, F32R), ("w2C", 65, 96, F32R), ("w3A", 97, 96, F32R),
        ("decA", 128, 64, BF16), ("decB", 128, 64, BF16),
        ("relA", 128, NSLOT * 24, BF16), ("relB", 128, NSLOT * 24, BF16),
        ("b_se", 64, 1, F32), ("b_c1", 64, 1, F32), ("b_c2", 32, 1, F32),
        ("b_c3", 32, 1, F32), ("c_d", 64, 1, F32), ("b_hp24", 24, 1, F32),
        ("ones", 1, B, F32R),
    ]:
        wd[name] = nc.dram_tensor(name, [p, f], dt, kind="ExternalInput")
    rels_d = nc.dram_tensor("rels", [24, B], F32, kind="ExternalOutput")

    with tile.TileContext(nc) as tc:
        with (
            tc.tile_pool(name="weights", bufs=1) as wpool,
            tc.tile_pool(name="rings", bufs=1) as rpool,
            tc.tile_pool(name="stage", bufs=3) as xpool,
            tc.tile_pool(name="pdec", bufs=PSUM_BUFS[0], space="PSUM") as pdec,
            tc.tile_pool(name="pc1", bufs=PSUM_BUFS[1], space="PSUM") as pc1,
            tc.tile_pool(name="pc2", bufs=PSUM_BUFS[2], space="PSUM") as pc2,
            tc.tile_pool(name="pc3", bufs=PSUM_BUFS[3], space="PSUM") as pc3,
        ):
            w = {k: wpool.tile_from(v[:], name=k)
                 for k, v in wd.items() if k != "ones"}

            obsA = rpool.tile([128, B], F32R, tag="obsA")   # ring slots 0,1
            obsC = rpool.tile([65, B], F32R, tag="obsC")    # slot 2 + ones row
            c1A = rpool.tile([128, B], F32R, tag="c1A")
            c1C = rpool.tile([65, B], F32R, tag="c1C")
            c2r = rpool.tile([97, B], F32R, tag="c2r")      # 3 bands + ones row
            S_all = rpool.tile([128, NSLOT, B], BF16, tag="S_all")
            MX_all = rpool.tile([128, NSLOT, NS], BF16, tag="MX_all")
            relout = rpool.tile([24, B], F32, tag="relout")

            # memset doesn't support float32r on HW ISA — DMA the ones rows
            nc.sync.dma_start(out=obsC[64:65, :], in_=wd["ones"][:])
            nc.sync.dma_start(out=c1C[64:65, :], in_=wd["ones"][:])
            nc.sync.dma_start(out=c2r[96:97, :], in_=wd["ones"][:])

            def obs_slot(j, ci):
                sl = slice(ci * CH, (ci + 1) * CH)
                if j == 0:
                    return obsA[0:64, sl]
                if j == 1:
                    return obsA[64:128, sl]
                return obsC[0:64, sl]

            def c1_slot(j, ci):
                sl = slice(ci * CH, (ci + 1) * CH)
                if j == 0:
                    return c1A[0:64, sl]
                if j == 1:
                    return c1A[64:128, sl]
                return c1C[0:64, sl]

            for g in range(T + SEQ - 1):           # g = 0..18
                if g < T:
                    xs = xpool.tile([3, B], F32R, tag="xs")
                    nc.sync.dma_start(out=xs[:], in_=obs_d[g])
                # chunk-pair-major emission: each pair runs its whole
                # stage chain before the next pair, tightening the
                # scheduler's cross-engine pipelining
                for cp in range(NCHUNK // 2):
                    for sub in range(2):
                        ci = 2 * cp + sub
                        sl = slice(ci * CH, (ci + 1) * CH)
                        # ---- stage 1: obs column g ----
                        ps = pdec.tile([64, CH], F32, tag="psdec")
                        if g < T:
                            nc.tensor.matmul(ps[:], w["wse_t"][:],
                                             xs[:, sl], start=True, stop=True)
                            nc.vector.tensor_copy(out=obs_slot(g % 3, ci),
                                                  in_=ps[:])
                        else:
                            s = g - T
                            band, slot = (s % 2) * 64, s // 2
                            nc.tensor.matmul(
                                ps[:], w["decA"][band:band + 64, :],
                                S_all[band:band + 64, slot, sl],
                                start=True, stop=False)
                            mxb = (MX_all[band:band + 64, slot,
                                          ci * (CH // SCENE):(ci + 1) * (CH // SCENE)]
                                   .unsqueeze(2).broadcast_to((64, CH // SCENE, SCENE)))
                            nc.tensor.matmul(ps[:], w["decB"][band:band + 64, :],
                                             mxb, start=False, stop=True)
                            nc.vector.tensor_add(obs_slot(g % 3, ci), ps[:],
                                                 w["c_d"][:].broadcast_to((64, CH)))
                        # ---- stage 2: conv1 ----
                        if g >= 2:
                            p = g - 2
                            r = p % 3
                            ps1 = pc1.tile([64, CH], F32, tag="psc1")
                            nc.tensor.matmul(ps1[:],
                                             w["w1A"][:, r * 64:(r + 1) * 64],
                                             obsA[:, sl], start=True, stop=False)
                            nc.tensor.matmul(ps1[:],
                                             w["w1C"][:, r * 64:(r + 1) * 64],
                                             obsC[:, sl], start=False, stop=True)
                            nc.scalar.activation(
                                c1_slot(p % 3, ci), ps1[:],
                                mybir.ActivationFunctionType.Relu)
                    # ---- stage 3: conv2 (wide pair) ----
                    if g >= 4:
                        q = g - 4
                        r = q % 3
                        band = (q % 3) * 32
                        ps2 = pc2.tile([32, 2 * CH], F32, tag="psc2")
                        for sub in range(2):
                            ci = 2 * cp + sub
                            sl = slice(ci * CH, (ci + 1) * CH)
                            half = ps2[:, sub * CH:(sub + 1) * CH]
                            nc.tensor.matmul(half,
                                             w["w2A"][:, r * 32:(r + 1) * 32],
                                             c1A[:, sl], start=True, stop=False)
                            nc.tensor.matmul(half,
                                             w["w2C"][:, r * 32:(r + 1) * 32],
                                             c1C[:, sl], start=False, stop=True)
                        slp = slice(2 * cp * CH, (2 * cp + 2) * CH)
                        nc.scalar.activation(c2r[band:band + 32, slp], ps2[:],
                                             mybir.ActivationFunctionType.Relu)
                    # ---- stage 4+5: conv3 and segmax ----
                    for sub in range(2):
                        ci = 2 * cp + sub
                        sl = slice(ci * CH, (ci + 1) * CH)
                        if g >= 6:
                            u = g - 6
                            r = u % 3
                            ps3 = pc3.tile([32, CH], F32, tag="psc3")
                            nc.tensor.matmul(ps3[:],
                                             w["w3A"][:, r * 32:(r + 1) * 32],
                                             c2r[:, sl], start=True, stop=True)
                            if u <= SEQ - 1:
                                b0 = (u % 2) * 64
                                if ci % 4 == 3:
                                    nc.vector.tensor_scalar_max(
                                        S_all[b0:b0 + 32, u // 2, sl],
                                        ps3[:], 0.0)
                                else:
                                    nc.scalar.activation(
                                        S_all[b0:b0 + 32, u // 2, sl], ps3[:],
                                        mybir.ActivationFunctionType.Relu)
                            if 1 <= u:
                                k = u - 1
                                b1_ = (k % 2) * 64 + 32
                                if u <= SEQ - 1:
                                    nc.vector.tensor_copy(
                                        out=S_all[b1_:b1_ + 32, k // 2, sl],
                                        in_=S_all[(u % 2) * 64:(u % 2) * 64 + 32,
                                                  u // 2, sl])
                                else:
                                    nc.vector.tensor_scalar_max(
                                        S_all[b1_:b1_ + 32, k // 2, sl],
                                        ps3[:], 0.0)
                        if g >= 7:
                            s = g - 7
                            band, slot = (s % 2) * 64, s // 2
                            nc.vector.reduce_max(
                                out=MX_all[band:band + 64, slot,
                                           ci * (CH // SCENE):(ci + 1) * (CH // SCENE)],
                                in_=S_all[band:band + 64, slot, sl]
                                .rearrange("p (s e) -> p s e", e=SCENE),
                                axis=mybir.AxisListType.X)

            # ---- endgame: rels for all 12 steps, M=24 block matmul ----
            if True:
                for ci in range(NCHUNK):
                    sl = slice(ci * CH, (ci + 1) * CH)
                    ps = pdec.tile([24, CH], F32, tag="psdec")
                    for slot in range(NSLOT):
                        nc.tensor.matmul(
                            ps[:], w["relA"][:, slot * 24:(slot + 1) * 24],
                            S_all[:, slot, sl],
                            start=(slot == 0), stop=False)
                        mxb = (MX_all[:, slot,
                                      ci * (CH // SCENE):(ci + 1) * (CH // SCENE)]
                               .unsqueeze(2).broadcast_to((128, CH // SCENE, SCENE)))
                        nc.tensor.matmul(ps[:],
                                         w["relB"][:, slot * 24:(slot + 1) * 24],
                                         mxb, start=False, stop=(slot == NSLOT - 1))
                    nc.scalar.activation(relout[:, sl], ps[:],
                                         mybir.ActivationFunctionType.Identity,
                                         bias=w["b_hp24"][:])
                nc.sync.dma_start(out=rels_d[:], in_=relout[:])

    nc.compile()   # bacc passes: split multi-waits into EventSemaphores etc.
    return nc


def _numpy_fallback(obs_traj, W_se, b_se, v1, g1, b1, v2, g2, b2, v3, g3, b3,
                    W_hp, b_hp, seq_start_end, seq_len):
    """Exact numpy implementation for inputs the device kernel wasn't built
    for (non-uniform segments / different seq_len)."""
    batch = obs_traj.shape[1]
    nseg = seq_start_end.shape[0]
    seg = np.searchsorted(seq_start_end[:, 0], np.arange(batch),
                          side="right") - 1

    def wn(v, g):
        n = np.sqrt((v * v).sum(axis=(1, 2)))
        return v * (g / n)[:, None, None]

    w1, w2, w3 = wn(v1, g1), wn(v2, g2), wn(v3, g3)

    def conv(x, w, b):
        O = w.shape[0]
        Tn = x.shape[2]
        out = np.zeros((x.shape[0], O, Tn - 2), np.float32)
        for t in range(Tn - 2):
            for k in range(3):
                out[:, :, t] += x[:, :, t + k] @ w[:, :, k].T
        return np.maximum(out + b[None, :, None], 0)

    emb = obs_traj @ W_se.T + b_se
    obs_emb = np.transpose(emb, (1, 2, 0)).copy()
    rels = []
    for _ in range(int(seq_len)):
        c3 = conv(conv(conv(obs_emb, w1, b1), w2, b2), w3, b3)
        s = c3.reshape(batch, 64)
        mx = np.full((nseg, 64), -np.inf, np.float32)
        np.maximum.at(mx, seg, s)
        st = np.concatenate([s, mx[seg]], axis=1)
        rel = st @ W_hp.T + b_hp
        dec = rel @ W_se.T + b_se
        obs_emb = np.concatenate([obs_emb[:, :, 1:], dec[:, :, None]], axis=2)
        rels.append(rel)
    return np.stack(rels).astype(np.float32)


def kernel(obs_traj, last_pos, last_pos_rel, W_se, b_se, v1, g1, b1,
           v2, g2, b2, v3, g3, b3, W_hp, b_hp, seq_start_end, seq_len):
    obs_traj = np.asarray(obs_traj, np.float32)
    seq_start_end = np.asarray(seq_start_end)
    args = [np.asarray(a, np.float32) for a in
            (W_se, b_se, v1, g1, b1, v2, g2, b2, v3, g3, b3, W_hp, b_hp)]

    starts = np.arange(BATCH // SCENE, dtype=np.int64) * SCENE
    uniform = (obs_traj.shape == (T, BATCH, 2)
               and int(seq_len) == SEQ
               and seq_start_end.shape == (BATCH // SCENE, 2)
               and np.array_equal(seq_start_end[:, 0], starts)
               and np.array_equal(seq_start_end[:, 1], starts + SCENE))
    if not uniform:
        return _numpy_fallback(obs_traj, *args, seq_start_end, seq_len)

    if "nc" not in _cache:
        _cache["nc"] = _build_module()
    nc = _cache["nc"]

    wdev = _host_weights(*args)
    obs_t = np.concatenate([obs_traj.transpose(0, 2, 1),
                            np.ones((T, 1, BATCH), np.float32)],
                           axis=1)  # (8, 3, 32768) with ones plane

    in_maps = []
    for core in range(NCORES):
        m = dict(wdev)
        m["obs"] = np.ascontiguousarray(obs_t[:, :, core * B:(core + 1) * B])
        in_maps.append(m)

    res = run_bass_kernel_spmd(nc, in_maps, core_ids=list(range(NCORES)))

    out = np.empty((SEQ, BATCH, 2), np.float32)
    for core in range(NCORES):
        arr = res.results[core]["rels"]          # (24, B)
        for c in range(2):
            out[:, core * B:(core + 1) * B, c] = arr[c::2]
    return out



# revision 37
# speedup vs baseline: 1.2377x; 1.2377x over previous
"""Trainium2 Bass kernel for nn_Encoder_66872640799015 (segment_reduce), v3.

Recurrent conv encoder over 32768 pedestrians (4096 scenes x 8), 12 steps.
Sharding: data-parallel over scenes - 8 cores x 4096 pedestrians, weights
replicated.

v3 structural changes vs the v2 baseline:
- W_se folded into conv1 (associativity): conv1 operates on RAW 3-channel
  (x, y, 1) columns; contraction is 9 rows -> ONE matmul per position
  (vs obs-embed matmul + 2 conv1 matmuls).  The obs embedding layer is gone;
  obs columns DMA directly into the column ring.
- The decoder feedback produces the 2-dim rel directly (out partitions 2).
  Ring columns ARE the rel outputs, so the 96-matmul rel endgame is deleted;
  rels stream to DRAM via one small DMA per step.
- b_hp is folded into conv1's ones-channel lhsT rows (boundary variants) and
  added back to the returned array on the host, keeping all PSUM drains
  bias-free and engine-assignable (ACT/DVE/Pool balance knobs).
- conv2/conv3 rings and weights in bf16 (err ~5.5e-3, tolerance 2e-2).
"""

import sys

sys.path.insert(0, "/opt/trn_rl_repo")

import numpy as np
import ml_dtypes

import concourse.bass as bass
import concourse.bacc as bacc
import concourse.tile as tile
from concourse import mybir
from concourse.bass_utils import run_bass_kernel_spmd

NCORES = 8
BATCH = 32768
B = BATCH // NCORES        # pedestrians per core
T = 8                      # obs_len
SEQ = 12                   # seq_len
SCENE = 8                  # pedestrians per scene
NS = B // SCENE            # scenes per core
CH = 512                   # free-dim chunk (one PSUM bank of fp32)
NCHUNK = B // CH
NSLOT = SEQ // 2           # S_all free slots (2 steps per slot)

F32 = mybir.dt.float32
F32R = mybir.dt.float32r
BF16 = mybir.dt.bfloat16

_cache = {}

# ---- engine-assignment knobs ----
# PSUM drains can only run on ACT ('a') or DVE ('v'): GPSIMD cannot access
# PSUM, nor run tensor-tensor max; segmax stays a DVE reduce.  All drains are
# wide [., 1024] (chunk-pair), PSUM pools hold one wide (2-bank) tile each.
R1_ENG = "aaaaavvv"   # relu1 (per pair if WIDE1 else per chunk)
R1P_ENG = "aaaaavvv"  # relu1 during prologue (no dec on ACT yet)
R3_ENG = "vvvvvvvv"   # relu3
DEC_ENG = "aaaaaaaa"  # dec drain (pure copy)
R2_ENG = "aaaa"       # relu2
DUP_POOL = 8          # units < DUP_POOL: dup copy on Pool instead of DVE
WIDE_DEC = False      # dec psum/drain wide [2,1024] vs narrow [2,512]
WIDE1 = False         # conv1 wide
WIDE2 = True          # conv2 wide
WIDE3 = False         # conv3/dup/segmax wide
PSUM_BUFS = (2, 2, 1, 2)   # bufs per pool (wide tiles cost 2 banks each)


def _host_weights(W_se, b_se, v1, g1, b1, v2, g2, b2, v3, g3, b3, W_hp, b_hp):
    """Derive all device weight tensors (pre-permuted / rotation variants)."""
    f32 = np.float32
    bf = ml_dtypes.bfloat16

    def wn(v, g):
        n = np.sqrt((v * v).sum(axis=(1, 2)))
        return (v * (g / n)[:, None, None]).astype(f32)

    w1 = wn(v1, g1)   # (64, 64, 3)
    w2 = wn(v2, g2)   # (32, 64, 3)
    w3 = wn(v3, g3)   # (32, 32, 3)

    # conv1 with W_se folded: taps act on raw (x, y, 1) columns.
    W1p = np.einsum("oik,ij->ojk", w1, W_se)           # (64, 2, 3)
    b1p = w1.sum(axis=2) @ b_se + b1                   # (64,)
    # b_hp correction for rel-columns (rel stored WITHOUT b_hp):
    bhp_corr = np.einsum("oik,i->ok", W1p, b_hp)       # (64, 3) per tap

    # Column-history ring R[67, B] (engine accesses must start at 32-aligned
    # partitions; DMA writes are exempt):
    #   rows  0: 3  rel slot 0 (cols c>=8 with (c-8)%3==0), ones at row 2
    #   rows  3:27  obs cols 0..7, 3-row pitch, ones at 3+3t+2
    #   rows 27:32  zero pad
    #   rows 32:35  rel slot 1, ones at 34
    #   rows 35:64  zero pad
    #   rows 64:67  rel slot 2, ones at 66
    # conv1 always contracts the full [0:67] window; unused rows carry zero
    # weights.  11 lhsT variants: p=0..7 boundary-specific, then 3 rotations.
    def col_row(c):
        return 3 + 3 * c if c <= 7 else 32 * ((c - 8) % 3)

    def conv1_lhst(p):
        out = np.zeros((67, 64), f32)
        bias = b1p.copy()
        for k in range(3):
            c = p + k
            if c >= 8:
                bias += bhp_corr[:, k]
            out[col_row(c):col_row(c) + 2, :] = W1p[:, :, k].T
        out[col_row(p) + 2, :] = bias   # tap-0 col's ones row carries bias
        return out

    w1v = np.stack([conv1_lhst(p) for p in range(8)]
                   + [conv1_lhst(8 + r) for r in range(3)], axis=1)
    # w1v: (67, 11, 64): variants 0-7 for p=0..7, 8-10 rotations for p>=8

    def conv_variants(w, nin, nout, nslots):
        out = np.zeros((nslots * nin, 3, nout), f32)
        for r in range(3):
            for j in range(nslots):
                k = (j - r) % 3
                out[j * nin:(j + 1) * nin, r, :] = w[:, :, k].T
        return out

    w2A = conv_variants(w2, 64, 32, 2)            # (128, 3, 32)
    w2C = conv_variants(w2, 64, 32, 3)[128:]      # (64, 3, 32)
    w3A = conv_variants(w3, 32, 32, 3)            # (96, 3, 32)
    # ring ones-rows carry conv biases (exact when b==0; bf16 otherwise)
    w2C = np.concatenate([w2C, np.tile(b2.reshape(1, 1, 32), (1, 3, 1))], 0)
    w3A = np.concatenate([w3A, np.tile(b3.reshape(1, 1, 32), (1, 3, 1))], 0)

    # dec: rel = W_hpa @ s + W_hpb @ mx[seg]   (b_hp folded/host-added)
    # S feature row (32*t + ch) -> reference feature index (2*ch + t)
    perm = np.array([2 * (r % 32) + r // 32 for r in range(64)])
    W_hpa, W_hpb = W_hp[:, :64], W_hp[:, 64:]
    decA = np.vstack([W_hpa[:, perm].T] * 2).copy()   # (128, 2) band-doubled
    decB = np.vstack([W_hpb[:, perm].T] * 2).copy()

    return {
        "w1v": w1v.reshape(67, 11 * 64),
        "w2A": w2A.reshape(128, 3 * 32).astype(bf),
        "w2C": w2C.reshape(65, 3 * 32).astype(bf),
        "w3A": w3A.reshape(97, 3 * 32).astype(bf),
        "decA": decA.astype(bf),
        "decB": decB.astype(bf),
        "onesb": np.ones((1, B), bf),
    }


def _ped_perm():
    """Within each 512-ped chunk: scene s member e -> offset e*64 + s."""
    idx = np.arange(B).reshape(-1, CH)                  # (chunks, 512)
    s, e = np.divmod(idx % CH, SCENE)                   # scene-in-chunk, member
    out = idx // CH * CH + e * (CH // SCENE) + s
    return out.reshape(-1)                              # perm: new[i] = old? see use


def _ring_init(obs_core):
    """Host-assembled initial ring image [67, B]: zeros, rel-slot ones rows,
    obs columns with their ones rows."""
    inv = np.argsort(_ped_perm())      # device slot j <- original ped inv[j]
    R = np.zeros((67, B), np.float32)
    R[2] = R[34] = R[66] = 1.0
    for t in range(T):
        R[3 + 3 * t:5 + 3 * t] = obs_core[t].T[:, inv]      # (2, B)
        R[5 + 3 * t] = 1.0
    return R


def _build_module():
    """Build the SPMD Bass module (input-independent, cached)."""
    nc = bacc.Bacc()

    obs_d = nc.dram_tensor("obs", [67, B], F32R, kind="ExternalInput")
    wd = {}
    for name, p, f, dt in [
        ("w1v", 67, 11 * 64, F32R),
        ("w2A", 128, 96, BF16), ("w2C", 65, 96, BF16), ("w3A", 97, 96, BF16),
        ("decA", 128, 2, BF16), ("decB", 128, 2, BF16),
        ("onesb", 1, B, BF16),
    ]:
        wd[name] = nc.dram_tensor(name, [p, f], dt, kind="ExternalInput")
    rels_d = nc.dram_tensor("rels", [24, B], F32R, kind="ExternalOutput")

    Relu = mybir.ActivationFunctionType.Relu
    Ident = mybir.ActivationFunctionType.Identity

    def drain(eng, out, in_, relu):
        """PSUM->SBUF drain on ACT or DVE, optionally with relu."""
        if eng == "a":
            nc.scalar.activation(out, in_, Relu if relu else Ident)
        elif relu:
            nc.vector.tensor_scalar_max(out, in_, 0.0)
        else:
            nc.vector.tensor_copy(out=out, in_=in_)

    with tile.TileContext(nc) as tc:
        with (
            tc.tile_pool(name="weights", bufs=1) as wpool,
            tc.tile_pool(name="rings", bufs=1) as rpool,
            tc.tile_pool(name="segt", bufs=3) as segp,
            tc.tile_pool(name="pdec", bufs=PSUM_BUFS[0], space="PSUM") as pdec,
            tc.tile_pool(name="pc1", bufs=PSUM_BUFS[1], space="PSUM") as pc1,
            tc.tile_pool(name="pc2", bufs=PSUM_BUFS[2], space="PSUM") as pc2,
            tc.tile_pool(name="pc3", bufs=PSUM_BUFS[3], space="PSUM") as pc3,
        ):
            # weights ride the ACT HWDGE queue so obs columns (SP queue)
            # aren't serialized behind them at startup
            w = {k: wpool.tile_from(v[:], name=k,
                                    forced_dma_engine=mybir.EngineType.Activation)
                 for k, v in wd.items() if k not in ("ones", "onesb")}

            ring = rpool.tile([67, B], F32R, tag="ring")    # column history
            c1A = rpool.tile([128, B], BF16, tag="c1A")     # slots 0,1
            c1C = rpool.tile([65, B], BF16, tag="c1C")      # slot 2 + ones
            c2r = rpool.tile([97, B], BF16, tag="c2r")      # 3 bands + ones
            S_all = rpool.tile([128, NSLOT, B], BF16, tag="S_all")
            MX_all = rpool.tile([128, NSLOT, NS], BF16, tag="MX_all")

            # single host-assembled ring image: zeros, ones rows, obs columns
            nc.sync.dma_start(out=ring[:], in_=obs_d[:])
            nc.sync.dma_start(out=c1C[64:65, :], in_=wd["onesb"][:])
            nc.sync.dma_start(out=c2r[96:97, :], in_=wd["onesb"][:])

            def _relu3_seg(u, unit, slx, p3):
                if u <= SEQ - 1:
                    b0 = (u % 2) * 64
                    drain(R3_ENG[unit], S_all[b0:b0 + 32, u // 2, slx],
                          p3, relu=True)
                if 1 <= u:
                    k = u - 1
                    b1_ = (k % 2) * 64 + 32
                    if u <= SEQ - 1:
                        eng = nc.gpsimd if unit < DUP_POOL else nc.vector
                        eng.tensor_copy(
                            out=S_all[b1_:b1_ + 32, k // 2, slx],
                            in_=S_all[(u % 2) * 64:(u % 2) * 64 + 32,
                                      u // 2, slx])
                    else:
                        nc.vector.tensor_scalar_max(
                            S_all[b1_:b1_ + 32, k // 2, slx], p3, 0.0)

            def c1_slot(j, sl):
                if j == 0:
                    return c1A[0:64, sl]
                if j == 1:
                    return c1A[64:128, sl]
                return c1C[0:64, sl]

            for g in range(T + SEQ):               # g = 0..19
                for cp in range(NCHUNK // 2):
                    sl2 = slice(2 * cp * CH, (2 * cp + 2) * CH)     # pair
                    # ---- stage 1: dec for step s = g-8 -> ring col g ----
                    if g >= T:
                        s = g - T
                        band, slot = (s % 2) * 64, s // 2
                        rb = 32 * ((g - 8) % 3)
                        if WIDE_DEC:
                            psd = pdec.tile([2, 2 * CH], F32, tag="psdec")
                        for sub in range(2):
                            ci = 2 * cp + sub
                            sl = slice(ci * CH, (ci + 1) * CH)
                            if WIDE_DEC:
                                pd = psd[:, sub * CH:(sub + 1) * CH]
                            else:
                                psd = pdec.tile([2, CH], F32, tag="psdec")
                                pd = psd[:]
                            nc.tensor.matmul(
                                pd, w["decA"][band:band + 64, :],
                                S_all[band:band + 64, slot, sl],
                                start=True, stop=False)
                            mxb = (MX_all[band:band + 64, slot,
                                          ci * (CH // SCENE):(ci + 1) * (CH // SCENE)]
                                   .unsqueeze(1).broadcast_to((64, SCENE, CH // SCENE)))
                            nc.tensor.matmul(pd, w["decB"][band:band + 64, :],
                                             mxb, start=False, stop=True)
                            if not WIDE_DEC:
                                drain(DEC_ENG[ci], ring[rb:rb + 2, sl],
                                      pd, relu=False)
                        if WIDE_DEC:
                            drain(DEC_ENG[cp], ring[rb:rb + 2, sl2],
                                  psd[:], relu=False)
                    # ---- stage 2: conv1 position p = g-2 ----
                    # prologue conv1 borrows the idle pdec PSUM ring (tags
                    # share a pool's bufs) and splits drains evenly since ACT
                    # has no dec work yet
                    if 2 <= g <= 18:
                        p = g - 2
                        var = p if p <= 7 else 8 + (p - 8) % 3
                        if g < T:
                            pool = pdec if cp % 2 == 0 else pc1
                            r1eng = R1P_ENG[cp]
                        else:
                            pool = pc1
                            r1eng = R1_ENG[cp]
                        tag1 = "psdec" if pool is pdec else "psc1"
                        if WIDE1:
                            ps1 = pool.tile([64, 2 * CH], F32, tag=tag1)
                        for sub in range(2):
                            ci = 2 * cp + sub
                            sl = slice(ci * CH, (ci + 1) * CH)
                            if WIDE1:
                                p1 = ps1[:, sub * CH:(sub + 1) * CH]
                            else:
                                ps1 = pool.tile([64, CH], F32, tag=tag1)
                                p1 = ps1[:]
                            nc.tensor.matmul(p1,
                                             w["w1v"][:, var * 64:(var + 1) * 64],
                                             ring[:, sl], start=True, stop=True)
                            if not WIDE1:
                                drain(R1P_ENG[ci] if g < T else R1_ENG[ci],
                                      c1_slot(p % 3, sl), p1, relu=True)
                        if WIDE1:
                            drain(r1eng, c1_slot(p % 3, sl2), ps1[:], relu=True)
                    # ---- stage 3: conv2 ----
                    if 4 <= g <= 18:
                        q = g - 4
                        r = q % 3
                        band = (q % 3) * 32
                        if WIDE2:
                            ps2 = pc2.tile([32, 2 * CH], F32, tag="psc2")
                        for sub in range(2):
                            ci = 2 * cp + sub
                            sl = slice(ci * CH, (ci + 1) * CH)
                            if WIDE2:
                                half = ps2[:, sub * CH:(sub + 1) * CH]
                            else:
                                ps2 = pc2.tile([32, CH], F32, tag="psc2")
                                half = ps2[:]
                            nc.tensor.matmul(half,
                                             w["w2A"][:, r * 32:(r + 1) * 32],
                                             c1A[:, sl], start=True, stop=False)
                            nc.tensor.matmul(half,
                                             w["w2C"][:, r * 32:(r + 1) * 32],
                                             c1C[:, sl], start=False, stop=True)
                            if not WIDE2:
                                drain(R2_ENG[ci % 8 if len(R2_ENG) > 4 else cp],
                                      c2r[band:band + 32, sl], half, relu=True)
                        if WIDE2:
                            drain(R2_ENG[cp], c2r[band:band + 32, sl2], ps2[:],
                                  relu=True)
                    # ---- stage 4+5: conv3, dup, segmax ----
                    if 6 <= g <= 18:
                        u = g - 6
                        r = u % 3
                        units3 = [(sl2, cp)] if WIDE3 else [
                            (slice(ci * CH, (ci + 1) * CH), ci)
                            for ci in (2 * cp, 2 * cp + 1)]
                        if WIDE3:
                            ps3 = pc3.tile([32, 2 * CH], F32, tag="psc3")
                        for sub in range(2):
                            ci = 2 * cp + sub
                            sl = slice(ci * CH, (ci + 1) * CH)
                            if WIDE3:
                                p3 = ps3[:, sub * CH:(sub + 1) * CH]
                            else:
                                ps3 = pc3.tile([32, CH], F32, tag="psc3")
                                p3 = ps3[:]
                            nc.tensor.matmul(p3,
                                             w["w3A"][:, r * 32:(r + 1) * 32],
                                             c2r[:, sl], start=True, stop=True)
                            if not WIDE3:
                                _relu3_seg(u, ci, sl, p3)
                        if WIDE3:
                            _relu3_seg(u, cp, sl2, ps3[:])
                    if 7 <= g <= 18:
                        s = g - 7
                        band, slot = (s % 2) * 64, s // 2
                        for cix in (2 * cp, 2 * cp + 1):
                            o = cix * CH
                            sb = S_all[band:band + 64, slot, :]
                            mxsl = slice(cix * (CH // SCENE),
                                         (cix + 1) * (CH // SCENE))
                            t1 = segp.tile([64, CH // 2], BF16, tag="t1")
                            t2 = segp.tile([64, CH // 4], BF16, tag="t2")
                            nc.vector.tensor_max(
                                t1[:], sb[:, o:o + 256], sb[:, o + 256:o + 512])
                            nc.vector.tensor_max(
                                t2[:], t1[:, 0:128], t1[:, 128:256])
                            nc.vector.tensor_max(
                                MX_all[band:band + 64, slot, mxsl],
                                t2[:, 0:64], t2[:, 64:128])
                if g >= T:
                    # stream step-s rels to DRAM from the freshly written col
                    s = g - T
                    rb = 32 * (s % 3)
                    nc.sync.dma_start(out=rels_d[2 * s:2 * s + 2, :],
                                      in_=ring[rb:rb + 2, :])

    nc.compile()
    return nc


def _numpy_fallback(obs_traj, W_se, b_se, v1, g1, b1, v2, g2, b2, v3, g3, b3,
                    W_hp, b_hp, seq_start_end, seq_len):
    """Exact numpy implementation for inputs the device kernel wasn't built
    for (non-uniform segments / different seq_len)."""
    batch = obs_traj.shape[1]
    nseg = seq_start_end.shape[0]
    seg = np.searchsorted(seq_start_end[:, 0], np.arange(batch),
                          side="right") - 1

    def wn(v, g):
        n = np.sqrt((v * v).sum(axis=(1, 2)))
        return v * (g / n)[:, None, None]

    w1, w2, w3 = wn(v1, g1), wn(v2, g2), wn(v3, g3)

    def conv(x, w, b):
        O = w.shape[0]
        Tn = x.shape[2]
        out = np.zeros((x.shape[0], O, Tn - 2), np.float32)
        for t in range(Tn - 2):
            for k in range(3):
                out[:, :, t] += x[:, :, t + k] @ w[:, :, k].T
        return np.maximum(out + b[None, :, None], 0)

    emb = obs_traj @ W_se.T + b_se
    obs_emb = np.transpose(emb, (1, 2, 0)).copy()
    rels = []
    for _ in range(int(seq_len)):
        c3 = conv(conv(conv(obs_emb, w1, b1), w2, b2), w3, b3)
        s = c3.reshape(batch, 64)
        mx = np.full((nseg, 64), -np.inf, np.float32)
        np.maximum.at(mx, seg, s)
        st = np.concatenate([s, mx[seg]], axis=1)
        rel = st @ W_hp.T + b_hp
        dec = rel @ W_se.T + b_se
        obs_emb = np.concatenate([obs_emb[:, :, 1:], dec[:, :, None]], axis=2)
        rels.append(rel)
    return np.stack(rels).astype(np.float32)


def kernel(obs_traj, last_pos, last_pos_rel, W_se, b_se, v1, g1, b1,
           v2, g2, b2, v3, g3, b3, W_hp, b_hp, seq_start_end, seq_len):
    obs_traj = np.asarray(obs_traj, np.float32)
    seq_start_end = np.asarray(seq_start_end)
    args = [np.asarray(a, np.float32) for a in
            (W_se, b_se, v1, g1, b1, v2, g2, b2, v3, g3, b3, W_hp, b_hp)]

    starts = np.arange(BATCH // SCENE, dtype=np.int64) * SCENE
    uniform = (obs_traj.shape == (T, BATCH, 2)
               and int(seq_len) == SEQ
               and seq_start_end.shape == (BATCH // SCENE, 2)
               and np.array_equal(seq_start_end[:, 0], starts)
               and np.array_equal(seq_start_end[:, 1], starts + SCENE))
    if not uniform:
        return _numpy_fallback(obs_traj, *args, seq_start_end, seq_len)

    if "nc" not in _cache:
        _cache["nc"] = _build_module()
    nc = _cache["nc"]

    wdev = _host_weights(*args)

    in_maps = []
    for core in range(NCORES):
        m = dict(wdev)
        m["obs"] = _ring_init(obs_traj[:, core * B:(core + 1) * B, :])
        in_maps.append(m)

    res = run_bass_kernel_spmd(nc, in_maps, core_ids=list(range(NCORES)))

    perm = _ped_perm()
    out = np.empty((SEQ, BATCH, 2), np.float32)
    for core in range(NCORES):
        arr = res.results[core]["rels"][:, perm]    # un-interleave
        for c in range(2):
            out[:, core * B:(core + 1) * B, c] = arr[c::2]
    out += args[12].reshape(1, 1, 2)             # b_hp added on host
    return out


# revision 42
# speedup vs baseline: 1.2635x; 1.0208x over previous
"""Trainium2 Bass kernel for nn_Encoder_66872640799015 (segment_reduce), v3.

Recurrent conv encoder over 32768 pedestrians (4096 scenes x 8), 12 steps.
Sharding: data-parallel over scenes - 8 cores x 4096 pedestrians, weights
replicated.

v3 structural changes vs the v2 baseline:
- W_se folded into conv1 (associativity): conv1 operates on RAW 3-channel
  (x, y, 1) columns; contraction is 9 rows -> ONE matmul per position
  (vs obs-embed matmul + 2 conv1 matmuls).  The obs embedding layer is gone;
  obs columns DMA directly into the column ring.
- The decoder feedback produces the 2-dim rel directly (out partitions 2).
  Ring columns ARE the rel outputs, so the 96-matmul rel endgame is deleted;
  rels stream to DRAM via one small DMA per step.
- b_hp is folded into conv1's ones-channel lhsT rows (boundary variants) and
  added back to the returned array on the host, keeping all PSUM drains
  bias-free and engine-assignable (ACT/DVE/Pool balance knobs).
- conv2/conv3 rings and weights in bf16 (err ~5.5e-3, tolerance 2e-2).
"""

import sys

sys.path.insert(0, "/opt/trn_rl_repo")

import numpy as np
import ml_dtypes

import concourse.bass as bass
import concourse.bacc as bacc
import concourse.tile as tile
from concourse import mybir
from concourse.bass_utils import run_bass_kernel_spmd

NCORES = 8
BATCH = 32768
B = BATCH // NCORES        # pedestrians per core
T = 8                      # obs_len
SEQ = 12                   # seq_len
SCENE = 8                  # pedestrians per scene
NS = B // SCENE            # scenes per core
CH = 512                   # free-dim chunk (one PSUM bank of fp32)
NCHUNK = B // CH
NSLOT = SEQ // 2           # S_all free slots (2 steps per slot)

F32 = mybir.dt.float32
F32R = mybir.dt.float32r
BF16 = mybir.dt.bfloat16

_cache = {}

# ---- engine-assignment knobs ----
# PSUM drains can only run on ACT ('a') or DVE ('v'): GPSIMD cannot access
# PSUM, nor run tensor-tensor max; segmax stays a DVE reduce.  All drains are
# wide [., 1024] (chunk-pair), PSUM pools hold one wide (2-bank) tile each.
R1_ENG = "aaaaavvv"   # relu1 (per pair if WIDE1 else per chunk)
R1P_ENG = "aavvvvvv"  # relu1 during prologue (no dec on ACT yet)
R3_ENG = "vvvvvvvv"   # relu3
DEC_ENG = "aaaaaaaa"  # dec drain (pure copy)
R2_ENG = "aaaa"       # relu2
DUP_POOL = 5          # units < DUP_POOL: dup copy on Pool instead of DVE
WIDE_DEC = False      # dec psum/drain wide [2,1024] vs narrow [2,512]
WIDE1 = False         # conv1 wide
WIDE2 = True          # conv2 wide
WIDE3 = False         # conv3/dup/segmax wide
PSUM_BUFS = (2, 2, 1, 2)   # bufs per pool (wide tiles cost 2 banks each)


def _host_weights(W_se, b_se, v1, g1, b1, v2, g2, b2, v3, g3, b3, W_hp, b_hp):
    """Derive all device weight tensors (pre-permuted / rotation variants)."""
    f32 = np.float32
    bf = ml_dtypes.bfloat16

    def wn(v, g):
        n = np.sqrt((v * v).sum(axis=(1, 2)))
        return (v * (g / n)[:, None, None]).astype(f32)

    w1 = wn(v1, g1)   # (64, 64, 3)
    w2 = wn(v2, g2)   # (32, 64, 3)
    w3 = wn(v3, g3)   # (32, 32, 3)

    # conv1 with W_se folded: taps act on raw (x, y, 1) columns.
    W1p = np.einsum("oik,ij->ojk", w1, W_se)           # (64, 2, 3)
    b1p = w1.sum(axis=2) @ b_se + b1                   # (64,)
    # b_hp correction for rel-columns (rel stored WITHOUT b_hp):
    bhp_corr = np.einsum("oik,i->ok", W1p, b_hp)       # (64, 3) per tap

    # Column-history ring R[67, B] (engine accesses must start at 32-aligned
    # partitions; DMA writes are exempt):
    #   rows  0: 3  rel slot 0 (cols c>=8 with (c-8)%3==0), ones at row 2
    #   rows  3:27  obs cols 0..7, 3-row pitch, ones at 3+3t+2
    #   rows 27:32  zero pad
    #   rows 32:35  rel slot 1, ones at 34
    #   rows 35:64  zero pad
    #   rows 64:67  rel slot 2, ones at 66
    # conv1 always contracts the full [0:67] window; unused rows carry zero
    # weights.  11 lhsT variants: p=0..7 boundary-specific, then 3 rotations.
    def col_row(c):
        return 3 + 3 * c if c <= 7 else 32 * ((c - 8) % 3)

    def conv1_lhst(p):
        out = np.zeros((67, 64), f32)
        bias = b1p.copy()
        for k in range(3):
            c = p + k
            if c >= 8:
                bias += bhp_corr[:, k]
            out[col_row(c):col_row(c) + 2, :] = W1p[:, :, k].T
        out[col_row(p) + 2, :] = bias   # tap-0 col's ones row carries bias
        return out

    w1v = np.stack([conv1_lhst(p) for p in range(8)]
                   + [conv1_lhst(8 + r) for r in range(3)], axis=1)
    # w1v: (67, 11, 64): variants 0-7 for p=0..7, 8-10 rotations for p>=8

    def conv_variants(w, nin, nout, nslots):
        out = np.zeros((nslots * nin, 3, nout), f32)
        for r in range(3):
            for j in range(nslots):
                k = (j - r) % 3
                out[j * nin:(j + 1) * nin, r, :] = w[:, :, k].T
        return out

    w2A = conv_variants(w2, 64, 32, 2)            # (128, 3, 32)
    w2C = conv_variants(w2, 64, 32, 3)[128:]      # (64, 3, 32)
    w3A = conv_variants(w3, 32, 32, 3)            # (96, 3, 32)
    # ring ones-rows carry conv biases (exact when b==0; bf16 otherwise)
    w2C = np.concatenate([w2C, np.tile(b2.reshape(1, 1, 32), (1, 3, 1))], 0)
    w3A = np.concatenate([w3A, np.tile(b3.reshape(1, 1, 32), (1, 3, 1))], 0)

    # dec: rel = W_hpa @ s + W_hpb @ mx[seg]   (b_hp folded/host-added)
    # S feature row (32*t + ch) -> reference feature index (2*ch + t)
    perm = np.array([2 * (r % 32) + r // 32 for r in range(64)])
    W_hpa, W_hpb = W_hp[:, :64], W_hp[:, 64:]
    decA = np.vstack([W_hpa[:, perm].T] * 2).copy()   # (128, 2) band-doubled
    decB = np.vstack([W_hpb[:, perm].T] * 2).copy()

    return {
        "w1v": w1v.reshape(67, 11 * 64),
        "w2A": w2A.reshape(128, 3 * 32).astype(bf),
        "w2C": w2C.reshape(65, 3 * 32).astype(bf),
        "w3A": w3A.reshape(97, 3 * 32).astype(bf),
        "decA": decA.astype(bf),
        "decB": decB.astype(bf),
        "onesb": np.ones((1, B), bf),
    }


def _ped_perm():
    """Within each 512-ped chunk: scene s member e -> offset e*64 + s."""
    idx = np.arange(B).reshape(-1, CH)                  # (chunks, 512)
    s, e = np.divmod(idx % CH, SCENE)                   # scene-in-chunk, member
    out = idx // CH * CH + e * (CH // SCENE) + s
    return out.reshape(-1)                              # perm: new[i] = old? see use


def _ring_init(obs_core):
    """Host-assembled initial ring image [67, B]: zeros, rel-slot ones rows,
    obs columns with their ones rows."""
    inv = np.argsort(_ped_perm())      # device slot j <- original ped inv[j]
    R = np.zeros((67, B), np.float32)
    R[2] = R[34] = R[66] = 1.0
    for t in range(T):
        R[3 + 3 * t:5 + 3 * t] = obs_core[t].T[:, inv]      # (2, B)
        R[5 + 3 * t] = 1.0
    return R


def _build_module():
    """Build the SPMD Bass module (input-independent, cached)."""
    nc = bacc.Bacc()

    obs_d = nc.dram_tensor("obs", [67, B], F32R, kind="ExternalInput")
    wd = {}
    for name, p, f, dt in [
        ("w1v", 67, 11 * 64, F32R),
        ("w2A", 128, 96, BF16), ("w2C", 65, 96, BF16), ("w3A", 97, 96, BF16),
        ("decA", 128, 2, BF16), ("decB", 128, 2, BF16),
        ("onesb", 1, B, BF16),
    ]:
        wd[name] = nc.dram_tensor(name, [p, f], dt, kind="ExternalInput")
    rels_d = nc.dram_tensor("rels", [24, B], F32R, kind="ExternalOutput")

    Relu = mybir.ActivationFunctionType.Relu
    Ident = mybir.ActivationFunctionType.Identity

    def drain(eng, out, in_, relu):
        """PSUM->SBUF drain on ACT or DVE, optionally with relu."""
        if eng == "a":
            nc.scalar.activation(out, in_, Relu if relu else Ident)
        elif relu:
            nc.vector.tensor_scalar_max(out, in_, 0.0)
        else:
            nc.vector.tensor_copy(out=out, in_=in_)

    with tile.TileContext(nc) as tc:
        with (
            tc.tile_pool(name="weights", bufs=1) as wpool,
            tc.tile_pool(name="rings", bufs=1) as rpool,
            tc.tile_pool(name="segt", bufs=3) as segp,
            tc.tile_pool(name="pdec", bufs=PSUM_BUFS[0], space="PSUM") as pdec,
            tc.tile_pool(name="pc1", bufs=PSUM_BUFS[1], space="PSUM") as pc1,
            tc.tile_pool(name="pc2", bufs=PSUM_BUFS[2], space="PSUM") as pc2,
            tc.tile_pool(name="pc3", bufs=PSUM_BUFS[3], space="PSUM") as pc3,
        ):
            # weights ride the ACT HWDGE queue so obs columns (SP queue)
            # aren't serialized behind them at startup
            w = {k: wpool.tile_from(v[:], name=k,
                                    forced_dma_engine=mybir.EngineType.Activation)
                 for k, v in wd.items() if k not in ("ones", "onesb")}

            ring = rpool.tile([67, B], F32R, tag="ring")    # column history
            c1A = rpool.tile([128, B], BF16, tag="c1A")     # slots 0,1
            c1C = rpool.tile([65, B], BF16, tag="c1C")      # slot 2 + ones
            c2r = rpool.tile([97, B], BF16, tag="c2r")      # 3 bands + ones
            S_all = rpool.tile([128, NSLOT, B], BF16, tag="S_all")
            MX_all = rpool.tile([128, NSLOT, NS], BF16, tag="MX_all")

            # single host-assembled ring image: zeros, ones rows, obs columns
            nc.sync.dma_start(out=ring[:], in_=obs_d[:])
            nc.sync.dma_start(out=c1C[64:65, :], in_=wd["onesb"][:])
            nc.sync.dma_start(out=c2r[96:97, :], in_=wd["onesb"][:])

            def _relu3_seg(u, unit, slx, p3):
                if u <= SEQ - 1:
                    b0 = (u % 2) * 64
                    drain(R3_ENG[unit], S_all[b0:b0 + 32, u // 2, slx],
                          p3, relu=True)
                if 1 <= u:
                    k = u - 1
                    b1_ = (k % 2) * 64 + 32
                    if u <= SEQ - 1:
                        eng = nc.gpsimd if unit < DUP_POOL else nc.vector
                        eng.tensor_copy(
                            out=S_all[b1_:b1_ + 32, k // 2, slx],
                            in_=S_all[(u % 2) * 64:(u % 2) * 64 + 32,
                                      u // 2, slx])
                    else:
                        nc.vector.tensor_scalar_max(
                            S_all[b1_:b1_ + 32, k // 2, slx], p3, 0.0)

            def c1_slot(j, sl):
                if j == 0:
                    return c1A[0:64, sl]
                if j == 1:
                    return c1A[64:128, sl]
                return c1C[0:64, sl]

            for g in range(T + SEQ):               # g = 0..19
                for cp in range(NCHUNK // 2):
                    sl2 = slice(2 * cp * CH, (2 * cp + 2) * CH)     # pair
                    # ---- stage 1: dec for step s = g-8 -> ring col g ----
                    if g >= T:
                        s = g - T
                        band, slot = (s % 2) * 64, s // 2
                        rb = 32 * ((g - 8) % 3)
                        if WIDE_DEC:
                            psd = pdec.tile([2, 2 * CH], F32, tag="psdec")
                        for sub in range(2):
                            ci = 2 * cp + sub
                            sl = slice(ci * CH, (ci + 1) * CH)
                            if WIDE_DEC:
                                pd = psd[:, sub * CH:(sub + 1) * CH]
                            else:
                                psd = pdec.tile([2, CH], F32, tag="psdec")
                                pd = psd[:]
                            nc.tensor.matmul(
                                pd, w["decA"][band:band + 64, :],
                                S_all[band:band + 64, slot, sl],
                                start=True, stop=False)
                            mxb = (MX_all[band:band + 64, slot,
                                          ci * (CH // SCENE):(ci + 1) * (CH // SCENE)]
                                   .unsqueeze(1).broadcast_to((64, SCENE, CH // SCENE)))
                            nc.tensor.matmul(pd, w["decB"][band:band + 64, :],
                                             mxb, start=False, stop=True)
                            if not WIDE_DEC:
                                de = DEC_ENG[ci]
                                drain(de, ring[rb:rb + 2, sl],
                                      pd, relu=False)
                        if WIDE_DEC:
                            drain(DEC_ENG[cp], ring[rb:rb + 2, sl2],
                                  psd[:], relu=False)
                    # ---- stage 2: conv1 position p = g-2 ----
                    # prologue conv1 borrows the idle pdec PSUM ring (tags
                    # share a pool's bufs) and splits drains evenly since ACT
                    # has no dec work yet
                    if 2 <= g <= 18:
                        p = g - 2
                        var = p if p <= 7 else 8 + (p - 8) % 3
                        if g < T:
                            pool = pdec if cp % 2 == 0 else pc1
                            r1eng = R1P_ENG[cp]
                        else:
                            pool = pc1
                            r1eng = R1_ENG[cp]
                        tag1 = "psdec" if pool is pdec else "psc1"
                        if WIDE1:
                            ps1 = pool.tile([64, 2 * CH], F32, tag=tag1)
                        for sub in range(2):
                            ci = 2 * cp + sub
                            sl = slice(ci * CH, (ci + 1) * CH)
                            if WIDE1:
                                p1 = ps1[:, sub * CH:(sub + 1) * CH]
                            else:
                                ps1 = pool.tile([64, CH], F32, tag=tag1)
                                p1 = ps1[:]
                            nc.tensor.matmul(p1,
                                             w["w1v"][:, var * 64:(var + 1) * 64],
                                             ring[:, sl], start=True, stop=True)
                            if not WIDE1:
                                drain(R1P_ENG[ci] if g < T else R1_ENG[ci],
                                      c1_slot(p % 3, sl), p1, relu=True)
                        if WIDE1:
                            drain(r1eng, c1_slot(p % 3, sl2), ps1[:], relu=True)
                    # ---- stage 3: conv2 ----
                    if 4 <= g <= 18:
                        q = g - 4
                        r = q % 3
                        band = (q % 3) * 32
                        if WIDE2:
                            ps2 = pc2.tile([32, 2 * CH], F32, tag="psc2")
                        for sub in range(2):
                            ci = 2 * cp + sub
                            sl = slice(ci * CH, (ci + 1) * CH)
                            if WIDE2:
                                half = ps2[:, sub * CH:(sub + 1) * CH]
                            else:
                                ps2 = pc2.tile([32, CH], F32, tag="psc2")
                                half = ps2[:]
                            nc.tensor.matmul(half,
                                             w["w2A"][:, r * 32:(r + 1) * 32],
                                             c1A[:, sl], start=True, stop=False)
                            nc.tensor.matmul(half,
                                             w["w2C"][:, r * 32:(r + 1) * 32],
                                             c1C[:, sl], start=False, stop=True)
                            if not WIDE2:
                                drain(R2_ENG[ci % 8 if len(R2_ENG) > 4 else cp],
                                      c2r[band:band + 32, sl], half, relu=True)
                        if WIDE2:
                            drain(R2_ENG[cp], c2r[band:band + 32, sl2], ps2[:],
                                  relu=True)
                    # ---- stage 4+5: conv3, dup, segmax ----
                    if 6 <= g <= 18:
                        u = g - 6
                        r = u % 3
                        units3 = [(sl2, cp)] if WIDE3 else [
                            (slice(ci * CH, (ci + 1) * CH), ci)
                            for ci in (2 * cp, 2 * cp + 1)]
                        if WIDE3:
                            ps3 = pc3.tile([32, 2 * CH], F32, tag="psc3")
                        for sub in range(2):
                            ci = 2 * cp + sub
                            sl = slice(ci * CH, (ci + 1) * CH)
                            if WIDE3:
                                p3 = ps3[:, sub * CH:(sub + 1) * CH]
                            else:
                                ps3 = pc3.tile([32, CH], F32, tag="psc3")
                                p3 = ps3[:]
                            nc.tensor.matmul(p3,
                                             w["w3A"][:, r * 32:(r + 1) * 32],
                                             c2r[:, sl], start=True, stop=True)
                            if not WIDE3:
                                _relu3_seg(u, ci, sl, p3)
                        if WIDE3:
                            _relu3_seg(u, cp, sl2, ps3[:])
                    if 7 <= g <= 18:
                        s = g - 7
                        band, slot = (s % 2) * 64, s // 2
                        for cix in (2 * cp, 2 * cp + 1):
                            o = cix * CH
                            sb = S_all[band:band + 64, slot, :]
                            mxsl = slice(cix * (CH // SCENE),
                                         (cix + 1) * (CH // SCENE))
                            t1 = segp.tile([64, CH // 2], BF16, tag="t1")
                            t2 = segp.tile([64, CH // 4], BF16, tag="t2")
                            nc.vector.tensor_max(
                                t1[:], sb[:, o:o + 256], sb[:, o + 256:o + 512])
                            nc.vector.tensor_max(
                                t2[:], t1[:, 0:128], t1[:, 128:256])
                            nc.vector.tensor_max(
                                MX_all[band:band + 64, slot, mxsl],
                                t2[:, 0:64], t2[:, 64:128])
                if g >= T:
                    # stream step-s rels to DRAM from the freshly written col
                    s = g - T
                    rb = 32 * (s % 3)
                    if g == T + SEQ - 1:
                        for cp in range(NCHUNK // 2):
                            sl2 = slice(2 * cp * CH, (2 * cp + 2) * CH)
                            nc.sync.dma_start(out=rels_d[2 * s:2 * s + 2, sl2],
                                              in_=ring[rb:rb + 2, sl2])
                    else:
                        nc.sync.dma_start(out=rels_d[2 * s:2 * s + 2, :],
                                          in_=ring[rb:rb + 2, :])

    nc.compile()
    return nc


def _numpy_fallback(obs_traj, W_se, b_se, v1, g1, b1, v2, g2, b2, v3, g3, b3,
                    W_hp, b_hp, seq_start_end, seq_len):
    """Exact numpy implementation for inputs the device kernel wasn't built
    for (non-uniform segments / different seq_len)."""
    batch = obs_traj.shape[1]
    nseg = seq_start_end.shape[0]
    seg = np.searchsorted(seq_start_end[:, 0], np.arange(batch),
                          side="right") - 1

    def wn(v, g):
        n = np.sqrt((v * v).sum(axis=(1, 2)))
        return v * (g / n)[:, None, None]

    w1, w2, w3 = wn(v1, g1), wn(v2, g2), wn(v3, g3)

    def conv(x, w, b):
        O = w.shape[0]
        Tn = x.shape[2]
        out = np.zeros((x.shape[0], O, Tn - 2), np.float32)
        for t in range(Tn - 2):
            for k in range(3):
                out[:, :, t] += x[:, :, t + k] @ w[:, :, k].T
        return np.maximum(out + b[None, :, None], 0)

    emb = obs_traj @ W_se.T + b_se
    obs_emb = np.transpose(emb, (1, 2, 0)).copy()
    rels = []
    for _ in range(int(seq_len)):
        c3 = conv(conv(conv(obs_emb, w1, b1), w2, b2), w3, b3)
        s = c3.reshape(batch, 64)
        mx = np.full((nseg, 64), -np.inf, np.float32)
        np.maximum.at(mx, seg, s)
        st = np.concatenate([s, mx[seg]], axis=1)
        rel = st @ W_hp.T + b_hp
        dec = rel @ W_se.T + b_se
        obs_emb = np.concatenate([obs_emb[:, :, 1:], dec[:, :, None]], axis=2)
        rels.append(rel)
    return np.stack(rels).astype(np.float32)


def kernel(obs_traj, last_pos, last_pos_rel, W_se, b_se, v1, g1, b1,
           v2, g2, b2, v3, g3, b3, W_hp, b_hp, seq_start_end, seq_len):
    obs_traj = np.asarray(obs_traj, np.float32)
    seq_start_end = np.asarray(seq_start_end)
    args = [np.asarray(a, np.float32) for a in
            (W_se, b_se, v1, g1, b1, v2, g2, b2, v3, g3, b3, W_hp, b_hp)]

    starts = np.arange(BATCH // SCENE, dtype=np.int64) * SCENE
    uniform = (obs_traj.shape == (T, BATCH, 2)
               and int(seq_len) == SEQ
               and seq_start_end.shape == (BATCH // SCENE, 2)
               and np.array_equal(seq_start_end[:, 0], starts)
               and np.array_equal(seq_start_end[:, 1], starts + SCENE))
    if not uniform:
        return _numpy_fallback(obs_traj, *args, seq_start_end, seq_len)

    if "nc" not in _cache:
        _cache["nc"] = _build_module()
    nc = _cache["nc"]

    wdev = _host_weights(*args)

    in_maps = []
    for core in range(NCORES):
        m = dict(wdev)
        m["obs"] = _ring_init(obs_traj[:, core * B:(core + 1) * B, :])
        in_maps.append(m)

    res = run_bass_kernel_spmd(nc, in_maps, core_ids=list(range(NCORES)))

    perm = _ped_perm()
    out = np.empty((SEQ, BATCH, 2), np.float32)
    for core in range(NCORES):
        arr = res.results[core]["rels"][:, perm]    # un-interleave
        for c in range(2):
            out[:, core * B:(core + 1) * B, c] = arr[c::2]
    out += args[12].reshape(1, 1, 2)             # b_hp added on host
    return out
